# revision 17
# baseline (speedup 1.0000x reference)
"""Trainium2 Bass kernel for the SE-attention block.

Math (per batch b):
    s[n]   = sum_c x[b,c,n]
    att[c] = sum_n x[b,c,n] * s[n]
    h      = relu(bn(W1 @ att))          (BN folded into scale/bias on host)
    a      = sigmoid(W2 @ h)
    out    = x[b] * a[:, None]

Sharding: data-parallel over batch B=16 across 8 cores (2 batches/core),
weights replicated, no collectives. HBM traffic is fixed at ~33.5 MB per
core (16.8 in + 16.8 out) -> ~82us at the ~410 GB/s a single HWDGE queue
sustains; everything else must hide under it.

Key structural choices:
  - x is declared float32r in DRAM (same bits as the fp32 input; dt.np
    maps both to np.float32).  The loads then satisfy the BIR rule that
    fp32r matmul operands come from an fp32r-producing instruction, and
    sB[:, chunk] = sum_t ones^T @ x_t accumulates directly in PSUM with
    fp32r matmuls (1 cycle/col vs fp32's 4).  The weights are exactly
    1.0, so fp32r only chops x mantissas (~tf32) inside the colsum;
    measured end-to-end rel-norm error ~5e-4 vs the 2e-2 gate.  This
    removes the old DVE/GpSimd adder tree entirely.
  - attq = rowsum(x * sB) fused scalar_tensor_tensor on DVE (the only
    engine that can do it: needs elementwise tensor*tensor + free-dim
    accum with PSUM operand), one per (quarter, tile), written into
    columns of a [128, 4] per-tile accumulator; W1 folds in once per
    tile at batch end (4 tiny matmuls per batch).
  - muls x*a: batch 0 on ACT+GpSimd (concurrent DVE work is stt =
    2-tensor single-port ops, which never contend with GpSimd; DVE
    tensor_scalar 2-port ops DO fully block GpSimd, so batch 1 -- where
    DVE does the muls -- uses DVE+ACT only).
  - All loads then all stores ride the SP HWDGE ring, issued by the SP
    sequencer (idle after load issue); loads and stores never contend.
    Const loads go on the ACT ring.
"""

import numpy as np

try:
    import concourse.bass as bass
except ImportError:  # fresh grading dir: repo not on sys.path
    import sys

    for p in ("/opt/trn_rl_repo", "/root/.axon_site/_ro/trn_rl_repo"):
        if p not in sys.path:
            sys.path.insert(0, p)
    import concourse.bass as bass

import concourse.tile as tile
from concourse import bacc, mybir
from concourse.bass_utils import run_bass_kernel_spmd

F32 = mybir.dt.float32
F32R = mybir.dt.float32r
AF = mybir.ActivationFunctionType
ALU = mybir.AluOpType

B, C, N = 16, 512, 4096
CR = 128          # squeeze dim C//4
NCORES = 8
BPC = B // NCORES  # batches per core
P = 128
CT = C // P        # channel tiles per batch
NCHUNK = 512       # matmul free-dim max (one psum bank)
BN_EPS = 1e-5

_nc_cache = None


def _build():
    nc = bacc.Bacc(None, target_bir_lowering=False)
    x = nc.declare_dram_parameter("x", [BPC, C, N], F32R, isOutput=False)
    w1t = nc.declare_dram_parameter("w1t", [C, CR], F32, isOutput=False)
    w2t = nc.declare_dram_parameter("w2t", [CR, C], F32, isOutput=False)
    bns = nc.declare_dram_parameter("bns", [CR, 1], F32, isOutput=False)
    bnb = nc.declare_dram_parameter("bnb", [CR, 1], F32, isOutput=False)
    y = nc.declare_dram_parameter("y", [BPC, C, N], F32, isOutput=True)

    NQ = N // 4   # 1024-wide pipeline quarters
    QS = 4        # quarters per batch

    with tile.TileContext(nc) as tc:
        with (
            tc.tile_pool(name="consts", bufs=1) as consts,
            tc.tile_pool(name="x", bufs=2 * CT * QS) as xpool,
            tc.tile_pool(name="big", bufs=1) as big,
            tc.tile_pool(name="small", bufs=4 * CT) as small,
            tc.tile_pool(name="psum", bufs=2, space="PSUM") as psum,
            tc.tile_pool(name="dram", bufs=4, space="DRAM") as dram,
        ):
            ones128 = consts.tile([P, P], F32)
            nc.vector.memset(ones128, 1.0)
            ones_r = consts.tile([P, P], F32R)
            nc.vector.tensor_copy(ones_r, ones128)
            # Const loads on the ACT HWDGE ring: x loads own the SP ring
            # from the first instruction.
            w1t_sb = consts.tile([P, CT, CR], F32)
            nc.scalar.dma_start(
                out=w1t_sb, in_=w1t[:].rearrange("(t p) o -> p t o", p=P)
            )
            w2t_sb = consts.tile([P, C], F32)
            nc.scalar.dma_start(out=w2t_sb, in_=w2t[:])
            bns_sb = consts.tile([P, 1], F32)
            nc.scalar.dma_start(out=bns_sb, in_=bns[:])
            bnb_sb = consts.tile([P, 1], F32)
            nc.scalar.dma_start(out=bnb_sb, in_=bnb[:])

            # Pre-clear const dependencies (tiny dummy consumers).
            scratch_ps = psum.tile([P, 1], F32, tag="mlp", name="scratch_ps")
            nc.tensor.matmul(
                scratch_ps, ones128, ones128[:, :1], start=True, stop=True
            )
            nc.tensor.matmul(
                scratch_ps, w1t_sb[:, 0, :], ones128[:, :1], start=True, stop=True
            )
            nc.tensor.matmul(
                scratch_ps, w2t_sb[:, :P], ones128[:, :1], start=True, stop=True
            )
            scratch_sb = consts.tile([P, 1], F32)
            nc.scalar.copy(scratch_sb, bns_sb)
            nc.scalar.copy(scratch_sb, bnb_sb)
            # Preload the sigmoid ACT table now: the lazy load costs 1.28us
            # right on the a(b0) critical path otherwise.
            nc.scalar.activation(scratch_sb, bns_sb, AF.Sigmoid)

            # Warm up the PE: the HAM clock gate keeps the array at 1.2GHz
            # until it sees ~3.4us of sustained activity, and the first sb
            # quarter otherwise runs cold right on the critical path.  The
            # warm tile takes one rotation of the sb-tag PSUM pool, freed
            # before the 4th real sb tile needs it.
            wsrc_f = consts.tile([P, NCHUNK], F32)
            nc.vector.memset(wsrc_f, 0.0)
            wsrc = consts.tile([P, NCHUNK], F32R)
            nc.vector.tensor_copy(wsrc, wsrc_f)
            warm = psum.tile([P, NQ], F32, tag="sb", bufs=3, name="warm")
            for j in range(10):
                cols = slice((j % 2) * NCHUNK, (j % 2 + 1) * NCHUNK)
                nc.tensor.matmul(
                    warm[:, cols], ones_r, wsrc, start=True, stop=True,
                )

            # Quarter-granular loads, all up front on the SP HWDGE ring, in
            # (batch, quarter) order so the first compute quarter is ready
            # ~7us in.  Tiles are fp32r-typed (same bits as fp32).
            xq = [
                [[None] * QS for _ in range(CT)] for _ in range(BPC)
            ]
            xqf = [
                [[None] * QS for _ in range(CT)] for _ in range(BPC)
            ]
            for b in range(BPC):
                for q in range(QS):
                    for t in range(CT):
                        tile_ = xpool.tile(
                            [P, NQ], F32R, tag="x", name=f"x_{b}_{t}_{q}"
                        )
                        # Alternate quarters across the two HWDGE rings so
                        # both queues feed the 16 SDMA engines.
                        ring = nc.sync if q % 2 == 0 else nc.scalar
                        ring.dma_start(
                            out=tile_,
                            in_=x[b, t * P : (t + 1) * P, q * NQ : (q + 1) * NQ],
                        )
                        xq[b][t][q] = tile_
                        xqf[b][t][q] = tile_.bitcast(F32)

            # attq[b][t] is a [128, QS] accumulator: column q holds
            # rowsum(x_t_q * sB_q).
            attq_all = [
                [
                    small.tile([P, QS], F32, tag="attq", name=f"attq_{b}_{t}")
                    for t in range(CT)
                ]
                for b in range(BPC)
            ]
            for b in range(BPC):
                for q in range(QS):
                    # sB[m, n] = s[n] broadcast to all 128 partitions:
                    # accumulate ones^T @ x_t over the 4 channel tiles in
                    # PSUM (fp32r, weights exactly 1.0).
                    sb = psum.tile(
                        [P, NQ], F32, tag="sb", bufs=3, name=f"sb_{b}_{q}"
                    )
                    for j in range(NQ // NCHUNK):
                        cols = slice(j * NCHUNK, (j + 1) * NCHUNK)
                        for t in range(CT):
                            nc.tensor.matmul(
                                sb[:, cols],
                                ones_r,
                                xq[b][t][q][:, cols],
                                start=(t == 0),
                                stop=(t == CT - 1),
                            )
                    for t in range(CT):
                        junk = big.tile(
                            [P, NQ], F32, tag="junk", bufs=1, name=f"junk_{b}_{q}_{t}"
                        )
                        # fused: junk = (x*1.0)*sb, attq col q = rowsum(junk)
                        nc.vector.scalar_tensor_tensor(
                            out=junk,
                            in0=xqf[b][t][q],
                            scalar=1.0,
                            in1=sb,
                            op0=ALU.mult,
                            op1=ALU.mult,
                            accum_out=attq_all[b][t][:, q : q + 1],
                        )

            for b in range(BPC):
                # h = relu(bn_scale * (W1 @ att) + bn_bias).  The sums over
                # quarters and tiles both fold into one PSUM accumulation:
                # hp4[o, q] = sum_t W1_t^T @ attq_t[:, q], then one ACT
                # copy-accum collapses the quarter axis before BN.
                hp4 = psum.tile([P, QS], F32, tag="mlp", name=f"hp4_{b}")
                for t in range(CT):
                    nc.tensor.matmul(
                        hp4,
                        w1t_sb[:, t, :],
                        attq_all[b][t],
                        start=(t == 0),
                        stop=(t == CT - 1),
                    )
                hjunk = small.tile([P, QS], F32, tag="hjunk", name=f"hjunk_{b}")
                hsum = small.tile([P, 1], F32, tag="hsum", name=f"hsum_{b}")
                nc.scalar.activation(hjunk, hp4, AF.Copy, accum_out=hsum)
                hb = small.tile([P, 1], F32, tag="hb", name=f"hb_{b}")
                nc.scalar.activation(
                    hb, hsum, AF.Relu, bias=bnb_sb, scale=bns_sb
                )

                # a = sigmoid(W2 @ h), per 128-channel chunk
                avec = []
                for t in range(CT):
                    apsum = psum.tile(
                        [P, 1], F32, tag="mlp", name=f"apsum_{b}_{t}"
                    )
                    nc.tensor.matmul(
                        apsum,
                        w2t_sb[:, t * P : (t + 1) * P],
                        hb,
                        start=True,
                        stop=True,
                    )
                    a_t = small.tile([P, 1], F32, tag="a", name=f"a_{b}_{t}")
                    nc.scalar.activation(a_t, apsum, AF.Sigmoid)
                    avec.append(a_t)

                # out = x * a; stores all issue from the (idle) SP
                # sequencer.  GpSimd never multiplies: its tensor_scalar
                # is a ~9ns/elem generic Q7 path AND it holds the shared
                # SBUF port, blocking DVE's 2-port muls.  Batch 0: all ACT
                # (DVE is mid stt-chain for batch 1).  Batch 1: DVE fast
                # tensor_scalar + ACT.
                last = b == BPC - 1
                idx = 0
                for t in range(CT):
                    for q in range(QS):
                        if last:
                            eng = nc.vector if idx % 2 == 0 else nc.scalar
                        else:
                            eng = nc.scalar
                        idx += 1
                        xt = xqf[b][t][q]
                        # Not in-place: the fp32r matmuls read the x tiles,
                        # and the BIR verifier rejects a second fp32-typed
                        # writer on that memory.
                        ot = big.tile(
                            [P, NQ], F32, tag="out", bufs=12,
                            name=f"o_{b}_{t}_{q}",
                        )
                        if eng is nc.scalar:
                            nc.scalar.mul(ot, xt, avec[t])
                        elif eng is nc.vector:
                            nc.vector.tensor_scalar_mul(ot, xt, avec[t])
                        else:
                            nc.gpsimd.tensor_scalar_mul(ot, xt, avec[t])
                        # Last batch: ACT-produced tiles store via the ACT
                        # ring (self-issued) so the final drain uses both
                        # rings; everything else via the idle SP sequencer.
                        sring = nc.scalar if (last and eng is nc.scalar) else nc.sync
                        sring.dma_start(
                            out=y[b, t * P : (t + 1) * P, q * NQ : (q + 1) * NQ],
                            in_=ot,
                        )
    return nc


def _get_nc():
    global _nc_cache
    if _nc_cache is None:
        _nc_cache = _build()
        if not _nc_cache.is_finalized():
            _nc_cache.finalize()
    return _nc_cache


def _host_prep(x, W1, gamma, beta, running_mean, running_var, W2):
    x = np.asarray(x, dtype=np.float32)
    rstd = 1.0 / np.sqrt(np.asarray(running_var, np.float32) + BN_EPS)
    bns = (np.asarray(gamma, np.float32) * rstd).reshape(CR, 1)
    bnb = (
        np.asarray(beta, np.float32)
        - np.asarray(running_mean, np.float32) * bns[:, 0]
    ).reshape(CR, 1)
    w1t = np.ascontiguousarray(np.asarray(W1, np.float32).T)  # [C, CR]
    w2t = np.ascontiguousarray(np.asarray(W2, np.float32).T)  # [CR, C]
    in_maps = []
    for c in range(NCORES):
        in_maps.append(
            {
                "x": np.ascontiguousarray(x[c * BPC : (c + 1) * BPC]),
                "w1t": w1t,
                "w2t": w2t,
                "bns": np.ascontiguousarray(bns, np.float32),
                "bnb": np.ascontiguousarray(bnb, np.float32),
            }
        )
    return in_maps


def _run(inputs, **spmd_kwargs):
    in_maps = _host_prep(**inputs)
    res = run_bass_kernel_spmd(
        _get_nc(), in_maps, list(range(NCORES)), **spmd_kwargs
    )
    out = np.concatenate([res.results[c]["y"] for c in range(NCORES)], axis=0)
    return out.astype(np.float32, copy=False), res


def kernel(**inputs):
    out, _ = _run(inputs)
    return out


# revision 22
# speedup vs baseline: 1.1078x; 1.1078x over previous
"""Trainium2 Bass kernel for the SE-attention block.

Math (per batch b):
    s[n]   = sum_c x[b,c,n]
    att[c] = sum_n x[b,c,n] * s[n]
    h      = relu(bn(W1 @ att))          (BN folded into scale/bias on host)
    a      = sigmoid(W2 @ h)
    out    = x[b] * a[:, None]

Sharding: data-parallel over batch B=16 across 8 cores (2 batches/core),
weights replicated, no collectives. HBM traffic is fixed at ~33.5 MB per
core (16.8 in + 16.8 out) -> ~80us at the ~420 GB/s the two HWDGE queues
sustain together; everything else must hide under it.

Key structural choices:
  - x is declared float32r in DRAM (same bits as the fp32 input; dt.np
    maps both to np.float32).  The loads then satisfy the BIR rule that
    fp32r matmul operands come from an fp32r-producing instruction, and
    sB[:, chunk] = sum_t ones^T @ x_t accumulates directly in PSUM with
    fp32r matmuls (1 cycle/col vs fp32's 4).  The weights are exactly
    1.0, so fp32r only chops x mantissas (~tf32) inside the colsum;
    measured end-to-end rel-norm error ~5e-4 vs the 2e-2 gate.  No
    adder tree: DVE's only att work is the 32 stt ops (fp32
    tensor_tensor-class ops are capped at 1x = 1.22us/tile there).
  - attq = rowsum(x * sB) fused scalar_tensor_tensor on DVE, written
    into columns of a [128, 4] per-tile accumulator.
  - The tiny MLP lives on the PE, which executes IN ORDER: its matmuls
    are emitted at hand-picked positions inside the sb stream so that
    (a) they are data-ready when the PE reaches them and (b) the ACT
    round-trips (BN, sigmoid) never stall the next sb quarter.
  - muls x*a: batch 0 all on ACT (DVE is mid stt-chain for batch 1;
    GpSimd's tensor_scalar is a slow generic Q7 path that also blocks
    DVE via the shared SBUF port pair -- never use it).  Batch 1
    alternates DVE (747ns 2x tensor_scalar) / ACT.
  - Loads alternate tiles across both HWDGE rings (SP + ACT) so both
    queues feed the 16 SDMA engines (~420 GB/s aggregate vs ~410 for
    one).  All stores issue from the idle SP sequencer except batch 1's
    ACT-produced tiles, which self-issue on the ACT ring for a
    two-ring final drain.
"""

import numpy as np

try:
    import concourse.bass as bass
except ImportError:  # fresh grading dir: repo not on sys.path
    import sys

    for p in ("/opt/trn_rl_repo", "/root/.axon_site/_ro/trn_rl_repo"):
        if p not in sys.path:
            sys.path.insert(0, p)
    import concourse.bass as bass

import concourse.tile as tile
from concourse import bacc, mybir
from concourse.bass_utils import run_bass_kernel_spmd

F32 = mybir.dt.float32
F32R = mybir.dt.float32r
AF = mybir.ActivationFunctionType
ALU = mybir.AluOpType

B, C, N = 16, 512, 4096
CR = 128          # squeeze dim C//4
NCORES = 8
BPC = B // NCORES  # batches per core
P = 128
CT = C // P        # channel tiles per batch
NCHUNK = 512       # matmul free-dim max (one psum bank)
BN_EPS = 1e-5

_nc_cache = None


def _build():
    nc = bacc.Bacc(None, target_bir_lowering=False)
    x = nc.declare_dram_parameter("x", [BPC, C, N], F32R, isOutput=False)
    w1t = nc.declare_dram_parameter("w1t", [C, CR], F32, isOutput=False)
    w2t = nc.declare_dram_parameter("w2t", [CR, C], F32, isOutput=False)
    bns = nc.declare_dram_parameter("bns", [CR, 1], F32, isOutput=False)
    bnb = nc.declare_dram_parameter("bnb", [CR, 1], F32, isOutput=False)
    y = nc.declare_dram_parameter("y", [BPC, C, N], F32, isOutput=True)

    NQ = N // 4   # 1024-wide pipeline quarters
    QS = 4        # quarters per batch

    with tile.TileContext(nc) as tc:
        with (
            tc.tile_pool(name="consts", bufs=1) as consts,
            tc.tile_pool(name="x", bufs=2 * CT * QS) as xpool,
            tc.tile_pool(name="big", bufs=1) as big,
            tc.tile_pool(name="small", bufs=4 * CT) as small,
            tc.tile_pool(name="psum", bufs=2, space="PSUM") as psum,
            tc.tile_pool(name="dram", bufs=4, space="DRAM") as dram,
        ):
            ones128 = consts.tile([P, P], F32)
            nc.vector.memset(ones128, 1.0)
            ones_r = consts.tile([P, P], F32R)
            nc.vector.tensor_copy(ones_r, ones128)
            # Const loads on the ACT HWDGE ring: x loads own the SP ring
            # from the first instruction.
            w1t_sb = consts.tile([P, CT, CR], F32)
            nc.scalar.dma_start(
                out=w1t_sb, in_=w1t[:].rearrange("(t p) o -> p t o", p=P)
            )
            w2t_sb = consts.tile([P, C], F32)
            nc.scalar.dma_start(out=w2t_sb, in_=w2t[:])
            bns_sb = consts.tile([P, 1], F32)
            nc.scalar.dma_start(out=bns_sb, in_=bns[:])
            bnb_sb = consts.tile([P, 1], F32)
            nc.scalar.dma_start(out=bnb_sb, in_=bnb[:])

            # Warm up the PE: the HAM clock gate keeps the array at 1.2GHz
            # until it sees ~3.4us of sustained activity, and the first sb
            # quarter otherwise runs cold right on the critical path.  The
            # warm tile takes one rotation of the sb-tag PSUM pool, freed
            # before the 4th real sb tile needs it.
            wsrc_f = consts.tile([P, NCHUNK], F32)
            nc.vector.memset(wsrc_f, 0.0)
            wsrc = consts.tile([P, NCHUNK], F32R)
            nc.vector.tensor_copy(wsrc, wsrc_f)
            warm = psum.tile([P, NQ], F32, tag="sb", bufs=3, name="warm")
            for j in range(10):
                cols = slice((j % 2) * NCHUNK, (j % 2 + 1) * NCHUNK)
                nc.tensor.matmul(
                    warm[:, cols], ones_r, wsrc, start=True, stop=True,
                )
            # Pre-clear const dependencies (tiny dummy consumers; PE order:
            # before the sb stream, ready by the time x(q0) lands).
            scratch_ps = psum.tile([P, 1], F32, tag="mlp", name="scratch_ps")
            nc.tensor.matmul(
                scratch_ps, ones128, ones128[:, :1], start=True, stop=True
            )
            nc.tensor.matmul(
                scratch_ps, w1t_sb[:, 0, :], ones128[:, :1], start=True, stop=True
            )
            nc.tensor.matmul(
                scratch_ps, w2t_sb[:, :P], ones128[:, :1], start=True, stop=True
            )

            # Quarter-granular loads in (batch, quarter) order, tiles
            # alternating across the two HWDGE rings.  After the first two
            # quarters' ACT-ring issues, slip in the sigmoid table preload
            # (the lazy 1.28us load otherwise lands on the a(b0) critical
            # path); scratch copies keep the bns/bnb loads early.
            xq = [[[None] * QS for _ in range(CT)] for _ in range(BPC)]
            xqf = [[[None] * QS for _ in range(CT)] for _ in range(BPC)]
            scratch_sb = consts.tile([P, 1], F32)
            for b in range(BPC):
                for q in range(QS):
                    if b == 0 and q == 2:
                        nc.scalar.activation(
                            scratch_sb, ones128[:, :1], AF.Sigmoid
                        )
                        nc.scalar.copy(scratch_sb, bns_sb)
                        nc.scalar.copy(scratch_sb, bnb_sb)
                    for t in range(CT):
                        tile_ = xpool.tile(
                            [P, NQ], F32R, tag="x", name=f"x_{b}_{t}_{q}"
                        )
                        ring = nc.sync if t % 2 == 0 else nc.scalar
                        ring.dma_start(
                            out=tile_,
                            in_=x[b, t * P : (t + 1) * P, q * NQ : (q + 1) * NQ],
                        )
                        xq[b][t][q] = tile_
                        xqf[b][t][q] = tile_.bitcast(F32)

            # attq[b][t] is a [128, QS] accumulator: column q holds
            # rowsum(x_t_q * sB_q).
            attq_all = [
                [
                    small.tile([P, QS], F32, tag="attq", name=f"attq_{b}_{t}")
                    for t in range(CT)
                ]
                for b in range(BPC)
            ]
            hp4s = [None] * BPC

            def emit_hp4(b):
                # hp4[o, q] = sum_t W1_t^T @ attq_t[:, q]: the sums over
                # tiles fold into one PSUM accumulation; the quarter axis
                # collapses on ACT before BN.
                hp4 = psum.tile([P, QS], F32, tag="mlp", name=f"hp4_{b}")
                for t in range(CT):
                    nc.tensor.matmul(
                        hp4,
                        w1t_sb[:, t, :],
                        attq_all[b][t],
                        start=(t == 0),
                        stop=(t == CT - 1),
                    )
                hp4s[b] = hp4

            def emit_ladder_and_muls(b):
                hjunk = small.tile([P, QS], F32, tag="hjunk", name=f"hjunk_{b}")
                hsum = small.tile([P, 1], F32, tag="hsum", name=f"hsum_{b}")
                nc.scalar.activation(hjunk, hp4s[b], AF.Copy, accum_out=hsum)
                hb = small.tile([P, 1], F32, tag="hb", name=f"hb_{b}")
                nc.scalar.activation(
                    hb, hsum, AF.Relu, bias=bnb_sb, scale=bns_sb
                )
                # a = sigmoid(W2 @ h): one [128, CT] PSUM tile, one matmul
                # per w2t chunk into its column, ONE sigmoid for all four.
                ap4 = psum.tile([P, CT], F32, tag="mlp", name=f"ap4_{b}")
                for t in range(CT):
                    nc.tensor.matmul(
                        ap4[:, t : t + 1],
                        w2t_sb[:, t * P : (t + 1) * P],
                        hb,
                        start=True,
                        stop=True,
                    )
                a4 = small.tile([P, CT], F32, tag="a", name=f"a4_{b}")
                nc.scalar.activation(a4, ap4, AF.Sigmoid)

                # out = x * a.  GpSimd never multiplies (slow generic Q7
                # path + it blocks DVE via the shared SBUF port).
                last = b == BPC - 1
                idx = 0
                for t in range(CT):
                    for q in range(QS):
                        eng = nc.vector if (last and idx % 2 == 0) else nc.scalar
                        idx += 1
                        ot = big.tile(
                            [P, NQ], F32, tag="out", bufs=12,
                            name=f"o_{b}_{t}_{q}",
                        )
                        # Not in-place: the fp32r matmuls read the x tiles,
                        # and the BIR verifier rejects a second fp32-typed
                        # writer on that memory.
                        if eng is nc.scalar:
                            nc.scalar.mul(ot, xqf[b][t][q], a4[:, t : t + 1])
                        else:
                            nc.vector.tensor_scalar_mul(
                                ot, xqf[b][t][q], a4[:, t : t + 1]
                            )
                        sring = nc.scalar if (last and eng is nc.scalar) else nc.sync
                        sring.dma_start(
                            out=y[b, t * P : (t + 1) * P, q * NQ : (q + 1) * NQ],
                            in_=ot,
                        )

            for b in range(BPC):
                for q in range(QS):
                    # Previous batch's MLP splits into the sb stream: hp4
                    # right after sb(b, 0) (its attq inputs are complete by
                    # then), the ACT ladder + ap4 one quarter later (hb is
                    # ready when the PE reaches ap4; the BN/sigmoid ACT
                    # round-trips overlap sb(b, 1)).
                    if b > 0 and q == 1:
                        emit_hp4(b - 1)
                    if b > 0 and q == 2:
                        emit_ladder_and_muls(b - 1)
                    # sB[m, n] = s[n] broadcast to all 128 partitions:
                    # accumulate ones^T @ x_t over the 4 channel tiles in
                    # PSUM (fp32r, weights exactly 1.0).
                    sb = psum.tile(
                        [P, NQ], F32, tag="sb", bufs=3, name=f"sb_{b}_{q}"
                    )
                    for j in range(NQ // NCHUNK):
                        cols = slice(j * NCHUNK, (j + 1) * NCHUNK)
                        for t in range(CT):
                            nc.tensor.matmul(
                                sb[:, cols],
                                ones_r,
                                xq[b][t][q][:, cols],
                                start=(t == 0),
                                stop=(t == CT - 1),
                            )
                    for t in range(CT):
                        junk = big.tile(
                            [P, NQ], F32, tag="junk", bufs=1, name=f"junk_{b}_{q}_{t}"
                        )
                        # fused: junk = (x*1.0)*sb, attq col q = rowsum(junk)
                        nc.vector.scalar_tensor_tensor(
                            out=junk,
                            in0=xqf[b][t][q],
                            scalar=1.0,
                            in1=sb,
                            op0=ALU.mult,
                            op1=ALU.mult,
                            accum_out=attq_all[b][t][:, q : q + 1],
                        )
            emit_hp4(BPC - 1)
            emit_ladder_and_muls(BPC - 1)

    return nc


def _get_nc():
    global _nc_cache
    if _nc_cache is None:
        _nc_cache = _build()
        if not _nc_cache.is_finalized():
            _nc_cache.finalize()
    return _nc_cache


def _host_prep(x, W1, gamma, beta, running_mean, running_var, W2):
    x = np.asarray(x, dtype=np.float32)
    rstd = 1.0 / np.sqrt(np.asarray(running_var, np.float32) + BN_EPS)
    bns = (np.asarray(gamma, np.float32) * rstd).reshape(CR, 1)
    bnb = (
        np.asarray(beta, np.float32)
        - np.asarray(running_mean, np.float32) * bns[:, 0]
    ).reshape(CR, 1)
    w1t = np.ascontiguousarray(np.asarray(W1, np.float32).T)  # [C, CR]
    w2t = np.ascontiguousarray(np.asarray(W2, np.float32).T)  # [CR, C]
    in_maps = []
    for c in range(NCORES):
        in_maps.append(
            {
                "x": np.ascontiguousarray(x[c * BPC : (c + 1) * BPC]),
                "w1t": w1t,
                "w2t": w2t,
                "bns": np.ascontiguousarray(bns, np.float32),
                "bnb": np.ascontiguousarray(bnb, np.float32),
            }
        )
    return in_maps


def _run(inputs, **spmd_kwargs):
    in_maps = _host_prep(**inputs)
    res = run_bass_kernel_spmd(
        _get_nc(), in_maps, list(range(NCORES)), **spmd_kwargs
    )
    out = np.concatenate([res.results[c]["y"] for c in range(NCORES)], axis=0)
    return out.astype(np.float32, copy=False), res


def kernel(**inputs):
    out, _ = _run(inputs)
    return out
